# revision 11
# baseline (speedup 1.0000x reference)
"""GAT-style 'cat' multi-head attention kernel for 8 TRN2 NeuronCores.

Data-parallel over batch: core b computes batch element b.

Identity: exp(leaky_relu(x)) = max(exp(x), exp(0.2 x)).  With scores
S[h,j,i] = sq[h,i] + sk[h,j] and 0/1 mask m[j,i]:

  p ~ m * max(Ek_j Eq_i, ek_j eq_i)        (Eq=e^sq, Ek=e^sk, ...)

Softmax is invariant to any per-(h,i) row scaling, so divide by
Eq_i*ek_j... specifically factor  E = Eq_i * ek_j * max(gk_j, hq_i)
with gk = e^{0.8 sk}, hq = e^{-0.8 sq}:
  - Eq_i row factor cancels between numerator and denominator
  - ek_j column factor is linear outside the max -> folded into v
so per head only  E' = m * max(gk_j-col, hq_i-row)  is materialized:
  W   = tensor_scalar(hqB max gk_j-col)       [DVE TS, per-partition max]
  W  += mb2 (0 / -30000 additive mask)        [SWDGE accumulate DMA]
  E'  = relu(W) = tensor_scalar(W max 0.0)    [DVE TS, immediate]
PV on TensorE: [ek*v_h | ek]^T @ E' -> (65, i) numer'+den' rows.
"""
import sys

sys.path.insert(0, "/opt/trn_rl_repo")

from contextlib import ExitStack

import numpy as np
import ml_dtypes

import concourse.bass as bass
import concourse.tile as tile
from concourse import bacc, mybir
from concourse.bass_utils import run_bass_kernel_spmd

F32 = mybir.dt.float32
BF16 = mybir.dt.bfloat16
Alu = mybir.AluOpType
Act = mybir.ActivationFunctionType

B, N, D, H, DK = 8, 1024, 512, 8, 64
ALPHA = 0.2
NJT = N // 128
NIT = N // 128
NKC = D // 128
NCH = D // 128

_CACHE = {}


def _build_nc():
    nc = bacc.Bacc("TRN2", target_bir_lowering=False, debug=False)

    def din(name, shape, dt):
        return nc.dram_tensor(name, shape, dt, kind="ExternalInput").ap()

    qT_d = din("qT", [D, N], BF16)
    kT_d = din("kT", [D, N], BF16)
    vT_d = din("vT", [D, N], BF16)
    mb2_d = din("mb2", [N, N], BF16)
    Cq_d = din("Cq", [D, H], BF16)
    Ck_d = din("Ck", [D, H], BF16)
    sqb_d = din("sqb", [1, H], BF16)
    skb_d = din("skb", [1, H], BF16)
    WvT_d = din("WvT", [D, D], BF16)
    WoT_d = din("WoT", [D, D], BF16)
    bv_d = din("bv", [1, D], BF16)
    bo_d = din("bo", [1, D], BF16)
    id8_d = din("id8", [8, 8], F32)
    id128_d = din("id128", [128, 128], F32)

    out_d = nc.dram_tensor("out", [N, D], F32, kind="ExternalOutput").ap()

    with tile.TileContext(nc) as tc, ExitStack() as ctx:
        consts = ctx.enter_context(tc.tile_pool(name="consts", bufs=1))
        t1pool = ctx.enter_context(tc.tile_pool(name="t1pool", bufs=3))
        small = ctx.enter_context(tc.tile_pool(name="small", bufs=4))
        single = ctx.enter_context(tc.tile_pool(name="single", bufs=1))
        dram = ctx.enter_context(tc.tile_pool(name="dram", bufs=1, space="DRAM"))
        ps_setup = ctx.enter_context(tc.tile_pool(name="ps_setup", bufs=2, space="PSUM"))
        ps_o = ctx.enter_context(tc.tile_pool(name="ps_o", bufs=4, space="PSUM"))
        ps_f = ctx.enter_context(tc.tile_pool(name="ps_f", bufs=2, space="PSUM"))

        # ---- input DMAs, ordered so setup matmuls can start early ----
        kT = consts.tile([128, NKC, N], BF16)
        for kc in range(NKC):
            nc.sync.dma_start(kT[:, kc, :], kT_d[kc * 128:(kc + 1) * 128, :])
        qT = consts.tile([128, NKC, N], BF16)
        for kc in range(NKC):
            nc.sync.dma_start(qT[:, kc, :], qT_d[kc * 128:(kc + 1) * 128, :])
        Ck = consts.tile([128, NKC, H], BF16)
        Cq = consts.tile([128, NKC, H], BF16)
        for kc in range(NKC):
            nc.sync.dma_start(Ck[:, kc, :], Ck_d[kc * 128:(kc + 1) * 128, :])
            nc.sync.dma_start(Cq[:, kc, :], Cq_d[kc * 128:(kc + 1) * 128, :])
        sqb = consts.tile([1, H], BF16)
        nc.sync.dma_start(sqb[:], sqb_d)
        skb = consts.tile([1, H], BF16)
        nc.sync.dma_start(skb[:], skb_d)
        id8 = consts.tile([8, 8], F32)
        nc.sync.dma_start(id8[:], id8_d)
        id128 = consts.tile([128, 128], F32)
        nc.sync.dma_start(id128[:], id128_d)

        # additive mask (0 unmasked / -30000 masked), resident in SBUF
        mb2 = consts.tile([128, NJT, N], BF16)
        for jt in range(NJT):
            nc.sync.dma_start(mb2[:, jt, :], mb2_d[jt * 128:(jt + 1) * 128, :])

        vT = consts.tile([128, NKC, N], BF16)
        WvT = consts.tile([128, NKC, D], BF16)
        for kc in range(NKC):
            nc.sync.dma_start(vT[:, kc, :], vT_d[kc * 128:(kc + 1) * 128, :])
            nc.sync.dma_start(WvT[:, kc, :], WvT_d[kc * 128:(kc + 1) * 128, :])
        bv = consts.tile([1, D], BF16)
        nc.sync.dma_start(bv[:], bv_d)
        WoT = consts.tile([128, NCH, D], BF16)
        for kc in range(NKC):
            nc.sync.dma_start(WoT[:, kc, :], WoT_d[kc * 128:(kc + 1) * 128, :])
        bo = consts.tile([1, D], BF16)
        nc.sync.dma_start(bo[:], bo_d)

        ones_row = consts.tile([1, N], BF16)
        nc.vector.memset(ones_row[:], 1.0)
        ones128 = consts.tile([1, 128], BF16)
        nc.vector.memset(ones128[:], 1.0)

        # ---- sk, sq score vectors: sk[h,j] = key_j @ Ck[:,h] + skb ----
        sk_f32 = single.tile([H, N], F32, tag="sk_f32")
        sq_bf = single.tile([H, N], BF16, tag="sq_bf")
        for (Cmat, rhsT, biasrow) in ((Ck, kT, skb), (Cq, qT, sqb)):
            for ih in range(2):
                ps = ps_setup.tile([H, 512], F32, tag="setup")
                sl = slice(ih * 512, (ih + 1) * 512)
                for kc in range(NKC):
                    nc.tensor.matmul(ps[:], Cmat[:, kc, :], rhsT[:, kc, sl],
                                     start=(kc == 0), stop=False)
                nc.tensor.matmul(ps[:], biasrow[:], ones_row[:, sl],
                                 start=False, stop=True)
                if Cmat is Cq:
                    nc.vector.tensor_copy(sq_bf[:, sl], ps[:])
                else:
                    nc.vector.tensor_copy(sk_f32[:, sl], ps[:])

        # hq = exp(-0.8 sq) rows -> DRAM scratch (row-broadcast source)
        hq_row = single.tile([H, N], BF16, tag="hq_row")
        nc.scalar.activation(hq_row[:], sq_bf[:], Act.Exp, bias=0.0, scale=-0.8)
        scr_hq = dram.tile([H, N], BF16)
        nc.sync.dma_start(scr_hq[:], hq_row[:])

        # gk = exp(0.8 sk), ek = exp(0.2 sk) as columns [128, NJT, H]
        ps_t = ps_setup.tile([128, NJT * H], F32, tag="setup")
        for t in range(NJT):
            nc.tensor.transpose(ps_t[:, t * H:(t + 1) * H],
                                sk_f32[:, t * 128:(t + 1) * 128], id8[:])
        gkT = consts.tile([128, NJT, H], F32)
        ekT = consts.tile([128, NJT, H], F32)
        nc.scalar.activation(gkT[:].rearrange("p a b -> p (a b)"), ps_t[:],
                             Act.Exp, bias=0.0, scale=0.8)
        nc.scalar.activation(ekT[:].rearrange("p a b -> p (a b)"), ps_t[:],
                             Act.Exp, bias=0.0, scale=ALPHA)

        # ---- v projection -> v_plus; then scale whole block by ek_j ----
        # v_plus[:, jt, h, 0:64] = ek_j * (v @ Wv)[j, (h,:)],  [.., 64] = ek_j
        v_plus = consts.tile([128, NJT, H, DK + 1], BF16)
        nc.vector.memset(v_plus[:, :, :, DK:DK + 1], 1.0)
        for jt in range(NJT):
            psv = ps_setup.tile([128, D], F32, tag="setup")
            jsl = slice(jt * 128, (jt + 1) * 128)
            for kc in range(NKC):
                nc.tensor.matmul(psv[:], vT[:, kc, jsl], WvT[:, kc, :],
                                 start=(kc == 0), stop=False)
            nc.tensor.matmul(psv[:], ones128[:], bv[:], start=False, stop=True)
            nc.vector.tensor_copy(v_plus[:, jt, :, 0:DK],
                                  psv[:].rearrange("p (h k) -> p h k", h=H))
            for h in range(H):
                if h % 2 == 0:
                    nc.vector.tensor_scalar(
                        v_plus[:, jt, h, :], v_plus[:, jt, h, :],
                        ekT[:, jt, h:h + 1], None, op0=Alu.mult)
                else:
                    nc.scalar.activation(
                        v_plus[:, jt, h, :], v_plus[:, jt, h, :],
                        Act.Identity, bias=0.0, scale=ekT[:, jt, h:h + 1])

        # ---- per-head attention ----
        scr_den = dram.tile([H, N], F32)
        xattnT = consts.tile([128, NCH, N], BF16)

        # hq row-broadcast tiles for all heads: [128, h, N] (~2MB total)
        hqB = consts.tile([128, H, N], BF16)
        nc.gpsimd.dma_start(
            hqB[:], scr_hq[:, :].unsqueeze(0).broadcast_to((128, H, N)))

        for h in range(H):
            # W = max(hq_i, gk_j)  (dual-operand TS, per jt tile)
            T1b = t1pool.tile([128, NJT, N], BF16, tag="t1")
            for jt in range(NJT):
                nc.vector.tensor_scalar(T1b[:, jt, :], hqB[:, h, :],
                                        gkT[:, jt, h:h + 1], None,
                                        op0=Alu.max)
            # W += additive mask (0 / -30000), then E' = relu(W)
            # (accumulate DMA is limited to ~512KB per op)
            for q in range(4):
                nc.gpsimd.dma_start(T1b[:, 2 * q:2 * q + 2, :],
                                    mb2[:, 2 * q:2 * q + 2, :],
                                    accum_op=Alu.add)
            flat = T1b[:].rearrange("p a b -> p (a b)")
            nc.vector.tensor_scalar(flat, flat, 0.0, None, op0=Alu.max)

            den_sb = small.tile([1, N], F32, tag="den")
            for ih in range(2):
                pso = ps_o.tile([65, 512], F32)
                nc.tensor.matmul(pso[0:1, 0:1], ones128[:, 0:1], ones128[:, 0:1],
                                 start=True, stop=True, skip_group_check=True)
                isl = slice(ih * 512, (ih + 1) * 512)
                for jt in range(NJT):
                    nc.tensor.matmul(pso[:], v_plus[:, jt, h, :],
                                     T1b[:, jt, isl],
                                     start=(jt == 0), stop=(jt == NJT - 1),
                                     skip_group_check=True)
                nc.scalar.copy(
                    xattnT[(h % 2) * 64:(h % 2) * 64 + 64, h // 2, isl],
                    pso[0:64, :])
                nc.scalar.copy(den_sb[:, isl], pso[64:65, :])
            nc.sync.dma_start(scr_den[h:h + 1, :], den_sb[:])

        # ---- reciprocal of denominators ----
        dens = single.tile([H, N], F32, tag="dens")
        nc.sync.dma_start(dens[:], scr_den[:])
        ps_dt = ps_setup.tile([128, NIT * H], F32, tag="setup")
        for t in range(NIT):
            nc.tensor.transpose(ps_dt[:, t * H:(t + 1) * H],
                                dens[:, t * 128:(t + 1) * 128], id8[:])
        recT = single.tile([128, NIT * H], F32, tag="recT")
        nc.vector.tensor_scalar(recT[:], ps_dt[:], 1e-30, None, op0=Alu.add)
        nc.vector.reciprocal(recT[:], recT[:])
        rec_back = single.tile([H, N], F32, tag="rec_back")
        for t in range(NIT):
            ps_b = ps_setup.tile([H, 128], F32, tag="setup")
            nc.tensor.transpose(ps_b[:], recT[:, t * H:(t + 1) * H], id128[:])
            nc.vector.tensor_copy(rec_back[:, t * 128:(t + 1) * 128], ps_b[:])
        scr_rec = dram.tile([H, N], F32)
        nc.sync.dma_start(scr_rec[:], rec_back[:])

        # ---- normalize + output projection ----
        rbpool = ctx.enter_context(tc.tile_pool(name="rbpool", bufs=2))
        for c in range(NCH):
            rb = rbpool.tile([128, N], F32)
            nc.sync.dma_start(rb[0:64, :],
                              scr_rec[2 * c:2 * c + 1, :].to_broadcast((64, N)))
            nc.sync.dma_start(rb[64:128, :],
                              scr_rec[2 * c + 1:2 * c + 2, :].to_broadcast((64, N)))
            nc.vector.tensor_mul(xattnT[:, c, :], xattnT[:, c, :], rb[:])

        for it in range(NIT):
            psf = ps_f.tile([128, D], F32)
            nc.tensor.matmul(psf[0:1, 0:1], ones128[:, 0:1], ones128[:, 0:1],
                             start=True, stop=True, skip_group_check=True)
            isl = slice(it * 128, (it + 1) * 128)
            for c in range(NCH):
                nc.tensor.matmul(psf[:], xattnT[:, c, isl], WoT[:, c, :],
                                 start=(c == 0), stop=False,
                                 skip_group_check=True)
            nc.tensor.matmul(psf[:], ones128[:], bo[:], start=False, stop=True,
                             skip_group_check=True)
            osb = small.tile([128, D], F32, tag="osb")
            nc.scalar.copy(osb[:], psf[:])
            nc.sync.dma_start(out_d[isl, :], osb[:])

    nc.compile()
    return nc


def _prep_host(query, key, value, mask, Wq, bq, Wk, bk, Wv, bv, Wo, bo, a):
    f32 = np.float32
    bf = ml_dtypes.bfloat16
    Aq = np.asarray(a, f32)[:, :DK]
    Ak = np.asarray(a, f32)[:, DK:]
    Wq = np.asarray(Wq, f32)
    Wk = np.asarray(Wk, f32)
    Cq = np.einsum("hkd,hk->dh", Wq.reshape(H, DK, D), Aq).astype(bf)
    Ck = np.einsum("hkd,hk->dh", Wk.reshape(H, DK, D), Ak).astype(bf)
    sqb = (np.asarray(bq, f32).reshape(H, DK) * Aq).sum(1).reshape(1, H).astype(bf)
    skb = (np.asarray(bk, f32).reshape(H, DK) * Ak).sum(1).reshape(1, H).astype(bf)

    shared = dict(
        Cq=np.ascontiguousarray(Cq), Ck=np.ascontiguousarray(Ck),
        sqb=sqb, skb=skb,
        WvT=np.ascontiguousarray(np.asarray(Wv, f32).T.astype(bf)),
        WoT=np.ascontiguousarray(np.asarray(Wo, f32).T.astype(bf)),
        bv=np.asarray(bv, f32).reshape(1, D).astype(bf),
        bo=np.asarray(bo, f32).reshape(1, D).astype(bf),
        id8=np.eye(8, dtype=f32),
        id128=np.eye(128, dtype=f32),
    )
    in_maps = []
    query = np.asarray(query, f32)
    key = np.asarray(key, f32)
    value = np.asarray(value, f32)
    mask = np.asarray(mask)
    for b in range(B):
        m = dict(shared)
        m["qT"] = np.ascontiguousarray(query[b].T.astype(bf))
        m["kT"] = np.ascontiguousarray(key[b].T.astype(bf))
        m["vT"] = np.ascontiguousarray(value[b].T.astype(bf))
        m["mb2"] = np.ascontiguousarray(
            ((mask[b].T.astype(f32) - 1.0) * 30000.0).astype(bf))
        in_maps.append(m)
    return in_maps


def kernel(query, key, value, mask, Wq, bq, Wk, bk, Wv, bv, Wo, bo, a):
    if "nc" not in _CACHE:
        _CACHE["nc"] = _build_nc()
    nc = _CACHE["nc"]
    in_maps = _prep_host(query, key, value, mask,
                         Wq, bq, Wk, bk, Wv, bv, Wo, bo, a)
    res = run_bass_kernel_spmd(nc, in_maps, core_ids=list(range(B)))
    out = np.stack([r["out"] for r in res.results], axis=0)
    return out.astype(np.float32)
